# revision 51
# baseline (speedup 1.0000x reference)
"""Bass/Trainium2 kernel for nn_NodesToEdges (gnn_message_passing).

out[b,i,j,:] = rms(edges[b,i,j,:])*g_e @ We + rms(nodes[b,i,:])*g_n @ Wr
             + rms(nodes[b,j,:])*g_n @ Wc + bias

Strategy: shard over i (rows) across 8 cores. The tiny node path
(row_proj / col_proj) is precomputed on host in numpy. The edge path
(134 MB in / 134 MB out, memory-bound) runs on device.

Per pair of 512-row blocks (layout [128=(a,pj), 512=(r,e)], j=pj*8+r):
  square[ACT] -> reduce[DVE] -> sqrt[ACT] -> recip[DVE]
  -> prescale[Pool] (x*inv -> bf16)
  -> 4x PE transpose (bf16) -> copy1[ACT] (psum->sbuf)
  -> PE matmuls DIRECT to row-major psum:
       rank-2 mm (ind2_rm stat, rp_mv moving) adds row_proj+bias,
       4 chunk mms (xT chunk stat, block-diag Wg moving) add edge term
  -> final[DVE]: psum + colrm(col_proj) -> outsb f32 -> DMA out.

Out-DMA issue alternates ACT/Pool queues; in-DMA on SP queue.
Stage offsets leave 2 engine-steps of slack per pipeline hop.
"""

import numpy as np

B, N, DE, DN = 2, 512, 64, 128
NCORES = 8
IPC = N // NCORES          # 64 i-rows per core
NBLK = B * IPC             # 128 blocks of 512 rows per core
NPAIR = NBLK // 2
EPS = float(np.finfo(np.float32).eps)


def _build_nc(npair=NPAIR):
    from contextlib import ExitStack

    import concourse.bass as bass
    import concourse.mybir as mybir

    f32 = mybir.dt.float32
    bf16 = mybir.dt.bfloat16
    SQRT = mybir.ActivationFunctionType.Sqrt
    SQUARE = mybir.ActivationFunctionType.Square

    nc = bass.Bass()
    nblk = 2 * npair
    x_d = nc.declare_dram_parameter("x", [nblk, N, DE], f32, isOutput=False)
    colrm_d = nc.declare_dram_parameter("colrm", [128, 2 * 512], f32, isOutput=False)
    rpmv_d = nc.declare_dram_parameter("rpmv", [2, NPAIR * 512], bf16, isOutput=False)
    ind2_d = nc.declare_dram_parameter("ind2", [2, 128], bf16, isOutput=False)
    wgblk_d = nc.declare_dram_parameter("wgblk", [128, 128], bf16, isOutput=False)
    id128_d = nc.declare_dram_parameter("id128", [128, 128], bf16, isOutput=False)
    out_d = nc.declare_dram_parameter("out", [nblk, N, DE], f32, isOutput=True)

    # stage offsets (engine-step lag per stage); sqrt/recip run PAIRED
    # (one [128,16] op per two iterations, at odd t).
    O_SQ, O_RED, O_SQT, O_REC = 0, 2, 3, 4
    O_PRE, O_FWD, O_CP1, O_MM, O_FIN, O_DMA = 6, 7, 8, 9, 10, 11
    NSTEP = npair + O_DMA + 1

    # buffer depths
    DX = 8    # xin slots, each holds TWO iterations [128, 1024]
    DQ = 6    # sq
    DSP = 4   # ssP / rmsP / invP paired tiles [128, 16]
    DXS = 6   # xs
    DXT = 6   # xT
    DO = 4    # outsb slots, each holds TWO iterations [128, 1024]
    DP1 = 4   # ps1 banks (bf16)
    DPO = 4   # ps_out banks (f32)

    st = ExitStack()
    with st:
        sb = lambda shape, dt, name: st.enter_context(
            nc.sbuf_tensor(name, shape, dt)
        )
        psum = lambda name, dt: st.enter_context(
            nc.psum_tensor(name, [128, 512], dt)
        )
        scratch1 = sb([128, 1], f32, "scratch1")
        colrm = sb([128, 1024], f32, "colrm_sb")
        rpmv = sb([2, NPAIR * 512], bf16, "rpmv_sb")
        ind2 = sb([2, 128], bf16, "ind2_sb")
        wgblk = sb([128, 128], bf16, "wgblk_sb")
        id128 = sb([128, 128], bf16, "id128_sb")
        epsb = sb([128, 1], f32, "epsb")
        xin = [sb([128, 1024], f32, f"xin{i}") for i in range(DX)]
        sq = [sb([128, 512], f32, f"sq{i}") for i in range(DQ)]
        ssP = [sb([128, 16], f32, f"ssP{i}") for i in range(DSP)]
        rmsP = [sb([128, 16], f32, f"rmsP{i}") for i in range(DSP)]
        invP = [sb([128, 16], f32, f"invP{i}") for i in range(DSP)]
        xs = [sb([128, 512], bf16, f"xs{i}") for i in range(DXS)]
        xT = [sb([128, 512], bf16, f"xT{i}") for i in range(DXT)]
        outsb = [sb([128, 1024], f32, f"outsb{i}") for i in range(DO)]
        ps1 = [psum(f"ps1{i}", bf16) for i in range(DP1)]
        pso = [psum(f"pso{i}", f32) for i in range(DPO)]

        sem = lambda name: st.enter_context(nc.semaphore(name))
        s_c = sem("s_c")
        s_in = [sem(f"s_in{i}") for i in range(DX)]
        s_out = [sem(f"s_out{i}") for i in range(DO)]
        s_pl = sem("s_pl")
        s_dve = sem("s_dve")
        s_act = sem("s_act")
        s_pe = sem("s_pe")

        # done[(stage, t)] = sem value once that op retired.
        done = {}
        for tp in range(npair // 2):
            done[("in2", tp)] = 16 * (tp // DX + 1)    # on s_in[tp % DX]
            done[("out2", tp)] = 16 * (tp // DO + 1)   # on s_out[tp % DO]
        # ACT: square(u), sqrt-pair at odd (u-O_SQT), copy1(u-O_CP1)
        c = 0
        for u in range(NSTEP):
            if u < npair:
                c += 1; done[("square", u)] = c
            tq = u - O_SQT
            if 0 <= tq < npair and tq % 2 == 1:
                c += 1; done[("sqrtp", tq // 2)] = c
            if O_CP1 <= u < npair + O_CP1:
                c += 1; done[("copy1", u - O_CP1)] = c
        # DVE: reduce(u-O_RED), recip-pair at odd (u-O_REC), final(u-O_FIN)
        c = 0
        for u in range(NSTEP):
            if O_RED <= u < npair + O_RED:
                c += 1; done[("reduce", u - O_RED)] = c
            tq = u - O_REC
            if 0 <= tq < npair and tq % 2 == 1:
                c += 1; done[("recipp", tq // 2)] = c
            if O_FIN <= u < npair + O_FIN:
                c += 1; done[("final", u - O_FIN)] = c
        # Pool: prescale(u-O_PRE)
        c = 0
        for u in range(NSTEP):
            if O_PRE <= u < npair + O_PRE:
                c += 1; done[("prescale", u - O_PRE)] = c
        # PE: fwdT(u-O_FWD) x4, mms(u-O_MM) x5
        c = 0
        for u in range(NSTEP):
            if O_FWD <= u < npair + O_FWD:
                c += 4; done[("fwdT", u - O_FWD)] = c
            if O_MM <= u < npair + O_MM:
                c += 5; done[("mm", u - O_MM)] = c

        CONST_TARGET = 5 * 16

        def in_src2(tp):
            return x_d[4 * tp : 4 * tp + 4].rearrange(
                "(g a) (p r) e -> (a p) g (r e)", a=2, p=64, r=8
            )

        def out_dst2(tp):
            return out_d[4 * tp : 4 * tp + 4].rearrange(
                "(g a) (p r) e -> (a p) g (r e)", a=2, p=64, r=8
            )

        def xin_sl(t):
            return xin[(t // 2) % DX][:, (t % 2) * 512 : (t % 2) * 512 + 512]

        def outsb_sl(t):
            return outsb[(t // 2) % DO][:, (t % 2) * 512 : (t % 2) * 512 + 512]

        def ssP_sl(t):
            return ssP[(t // 2) % DSP][:, (t % 2) * 8 : (t % 2) * 8 + 8]

        def invP_sl(t):
            return invP[(t // 2) % DSP][:, (t % 2) * 8 : (t % 2) * 8 + 8]

        with nc.Block() as block:

            @block.sync
            def _(sync):
                def dma_in(tp):
                    if tp >= DX:
                        sync.wait_ge(s_pl, done[("prescale", 2 * (tp - DX) + 1)])
                    sync.dma_start(
                        out=xin[tp % DX][:].rearrange("p (g x) -> p g x", g=2),
                        in_=in_src2(tp),
                    ).then_inc(s_in[tp % DX], 16)

                dma_in(0)  # first input before consts: unblocks the front ASAP
                for cdst, csrc in (
                    (rpmv, rpmv_d[:]),      # PE needs these 4 (s_c >= 64)
                    (ind2, ind2_d[:]),
                    (wgblk, wgblk_d[:]),
                    (id128, id128_d[:]),
                    (colrm, colrm_d[:]),    # only final(0) needs this (>= 80)
                ):
                    sync.dma_start(out=cdst[:], in_=csrc).then_inc(s_c, 16)
                for tp in range(1, npair // 2):
                    dma_in(tp)

            @block.scalar
            def _(scalar):
                # dummy op: trigger the lazy ACT table load before inputs land
                nc.scalar.activation(scratch1[:], scratch1[:], SQUARE)
                for u in range(NSTEP):
                    if u < npair:
                        t = u
                        scalar.wait_ge(s_in[(t // 2) % DX], done[("in2", t // 2)])
                        if t >= DQ:
                            scalar.wait_ge(s_dve, done[("reduce", t - DQ)])
                        nc.scalar.activation(
                            sq[t % DQ][:], xin_sl(t), SQUARE
                        ).then_inc(s_act, 1)
                    tq = u - O_SQT
                    if 0 <= tq < npair and tq % 2 == 1:
                        k = tq // 2
                        w = done[("reduce", tq)]
                        if k >= DSP:
                            w = max(w, done[("recipp", k - DSP)])
                        scalar.wait_ge(s_dve, w)
                        nc.scalar.activation(
                            rmsP[k % DSP][:], ssP[k % DSP][:], SQRT,
                            bias=epsb[:], scale=1.0 / DE,
                        ).then_inc(s_act, 1)
                    if O_CP1 <= u < npair + O_CP1:
                        t = u - O_CP1
                        w = done[("fwdT", t)]
                        if t >= DXT:
                            w = max(w, done[("mm", t - DXT)])
                        scalar.wait_ge(s_pe, w)
                        nc.scalar.copy(xT[t % DXT][:], ps1[t % DP1][:]).then_inc(
                            s_act, 1
                        )
                    if O_DMA <= u < npair + O_DMA:
                        t = u - O_DMA
                        if t % 4 == 1:
                            tp = t // 2
                            scalar.wait_ge(s_dve, done[("final", t)])
                            scalar.dma_start(
                                out=out_dst2(tp),
                                in_=outsb[tp % DO][:].rearrange(
                                    "p (g x) -> p g x", g=2
                                ),
                            ).then_inc(s_out[tp % DO], 16)

            @block.vector
            def _(vector):
                nc.vector.memset(epsb[:], EPS)
                for u in range(NSTEP):
                    if O_RED <= u < npair + O_RED:
                        t = u - O_RED
                        w = done[("square", t)]
                        if t // 2 >= DSP:
                            w = max(w, done[("sqrtp", t // 2 - DSP)])
                        vector.wait_ge(s_act, w)
                        nc.vector.tensor_reduce(
                            ssP_sl(t),
                            sq[t % DQ][:].rearrange("p (g e) -> p g e", e=DE),
                            axis=mybir.AxisListType.X,
                            op=mybir.AluOpType.add,
                        ).then_inc(s_dve, 1)
                    tq = u - O_REC
                    if 0 <= tq < npair and tq % 2 == 1:
                        k = tq // 2
                        vector.wait_ge(s_act, done[("sqrtp", k)])
                        if k >= DSP:
                            vector.wait_ge(
                                s_pl, done[("prescale", 2 * (k - DSP) + 1)]
                            )
                        nc.vector.reciprocal(
                            invP[k % DSP][:], rmsP[k % DSP][:]
                        ).then_inc(s_dve, 1)
                    if O_FIN <= u < npair + O_FIN:
                        t = u - O_FIN
                        if t == 0:
                            vector.wait_ge(s_c, CONST_TARGET)  # colrm loaded
                        vector.wait_ge(s_pe, done[("mm", t)])
                        if t // 2 >= DO:
                            vector.wait_ge(
                                s_out[(t // 2 - DO) % DO],
                                done[("out2", t // 2 - DO)],
                            )
                        bslice = 512 * ((2 * t) // IPC)
                        nc.vector.tensor_add(
                            outsb_sl(t),
                            pso[t % DPO][:],
                            colrm[:, bslice : bslice + 512],
                        ).then_inc(s_dve, 1)

            @block.gpsimd
            def _(pool):
                for u in range(NSTEP):
                    if O_PRE <= u < npair + O_PRE:
                        t = u - O_PRE
                        pool.wait_ge(s_dve, done[("recipp", t // 2)])
                        if t >= DXS:
                            pool.wait_ge(s_pe, done[("fwdT", t - DXS)])
                        nc.gpsimd.tensor_mul(
                            xs[t % DXS][:].rearrange("p (g e) -> p g e", e=DE),
                            xin_sl(t).rearrange("p (g e) -> p g e", e=DE),
                            invP_sl(t).unsqueeze(-1).broadcast_to([128, 8, DE]),
                        ).then_inc(s_pl, 1)
                    if O_DMA <= u < npair + O_DMA:
                        t = u - O_DMA
                        if t % 4 == 3:
                            tp = t // 2
                            pool.wait_ge(s_dve, done[("final", t)])
                            pool.dma_start(
                                out=out_dst2(tp),
                                in_=outsb[tp % DO][:].rearrange(
                                    "p (g x) -> p g x", g=2
                                ),
                            ).then_inc(s_out[tp % DO], 16)

            @block.tensor
            def _(tensor):
                tensor.wait_ge(s_c, 4 * 16)  # rpmv/ind2/wgblk/id128 loaded
                for u in range(NSTEP):
                    if O_FWD <= u < npair + O_FWD:
                        t = u - O_FWD
                        tensor.wait_ge(s_pl, done[("prescale", t)])
                        if t >= DP1:
                            tensor.wait_ge(s_act, done[("copy1", t - DP1)])
                        for q in range(4):
                            nc.tensor.transpose(
                                ps1[t % DP1][:, 128 * q : 128 * q + 128],
                                xs[t % DXS][:, 128 * q : 128 * q + 128],
                                id128[:],
                            ).then_inc(s_pe, 1)
                    if O_MM <= u < npair + O_MM:
                        t = u - O_MM
                        tensor.wait_ge(s_act, done[("copy1", t)])
                        if t >= DPO:
                            tensor.wait_ge(s_dve, done[("final", t - DPO)])
                        nc.tensor.matmul(
                            pso[t % DPO][:], ind2[:],
                            rpmv[:, 512 * t : 512 * t + 512],
                            start=True, stop=False,
                            skip_group_check=True,
                        ).then_inc(s_pe, 1)
                        for q in range(4):
                            nc.tensor.matmul(
                                pso[t % DPO][:, 128 * q : 128 * q + 128],
                                xT[t % DXT][:, 128 * q : 128 * q + 128],
                                wgblk[:],
                                start=False, stop=(q == 3),
                                skip_group_check=True,
                            ).then_inc(s_pe, 1)

    return nc


_NC_CACHE = {}


def _get_nc():
    if "nc" not in _NC_CACHE:
        _NC_CACHE["nc"] = _build_nc()
    return _NC_CACHE["nc"]


def _make_in_maps(edges, nodes, g_node, g_edge, W, b):
    import ml_dtypes

    edges = np.ascontiguousarray(edges, dtype=np.float32)
    nodes = np.ascontiguousarray(nodes, dtype=np.float32)

    # ---- host: tiny node path (B*N*dn = 131K elems)
    ms = np.mean(np.square(nodes), axis=-1, keepdims=True)
    nodes_n = nodes / np.sqrt(ms + EPS) * g_node  # [B, N, 128]
    Wr, Wc, We = W[:DN], W[DN : 2 * DN], W[2 * DN :]
    row_proj = (nodes_n @ Wr).astype(np.float32)  # [B, N, 64]
    col_proj = (nodes_n @ Wc).astype(np.float32)  # [B, N, 64]
    Wg = (g_edge[:, None] * We).astype(np.float32)  # fold g_edge into We

    # colrm[c, 512b + (o,e)] = col_proj[b, 8*(c%64)+o, e]  (bias lives in rpmv)
    cp = col_proj.astype(np.float32).reshape(B, 64, 8 * DE)  # [2, 64, 512]
    colrm = np.concatenate([cp, cp], axis=1)  # [2, 128, 512]
    colrm = np.ascontiguousarray(colrm.transpose(1, 0, 2)).reshape(128, 2 * 512)

    # block-diagonal Wg (even rows top-left, odd rows bottom-right)
    wgblk = np.zeros((128, 128), dtype=np.float32)
    wgblk[:64, :64] = Wg
    wgblk[64:, 64:] = Wg
    wgblk = wgblk.astype(ml_dtypes.bfloat16)
    id128 = np.eye(128, dtype=ml_dtypes.bfloat16)
    # ind2_rm[k, (a,pj)] = 1 if a == k  (stationary of the rank-2 rp matmul)
    ind2 = np.zeros((2, 128), dtype=ml_dtypes.bfloat16)
    ind2[0, :64] = 1.0
    ind2[1, 64:] = 1.0

    in_maps = []
    for c in range(NCORES):
        xsl = edges[:, c * IPC : (c + 1) * IPC]  # [B, 64, 512, 64]
        xsl = np.ascontiguousarray(xsl).reshape(NBLK, N, DE)
        # rpmv[k, 512t + 64r + e] = row_proj[blk=2t+k][e] + b[e]
        rp = row_proj[:, c * IPC : (c + 1) * IPC].reshape(NBLK, DE) + b  # [128, 64]
        rp = rp.astype(np.float32)
        rpmv = np.empty((2, NPAIR, 8, DE), dtype=np.float32)
        rpmv[0] = rp[0::2, None, :]
        rpmv[1] = rp[1::2, None, :]
        rpmv = rpmv.reshape(2, NPAIR * 512).astype(ml_dtypes.bfloat16)
        in_maps.append(
            {
                "x": xsl,
                "colrm": colrm,
                "rpmv": rpmv,
                "ind2": ind2,
                "wgblk": wgblk,
                "id128": id128,
            }
        )
    return in_maps


_LAST = None  # last BassKernelResults (exec_time_ns/trace when BASS_TRACE=1)


def kernel(edges, nodes, g_node, g_edge, W, b):
    in_maps = _make_in_maps(edges, nodes, g_node, g_edge, W, b)

    from concourse.bass_utils import run_bass_kernel_spmd

    nc = _get_nc()
    res = run_bass_kernel_spmd(nc, in_maps, list(range(NCORES)))
    global _LAST
    _LAST = res

    out = np.empty((B, N, N, DE), dtype=np.float32)
    for c in range(NCORES):
        oc = res.results[c]["out"].reshape(B, IPC, N, DE)
        out[:, c * IPC : (c + 1) * IPC] = oc
    return out


if __name__ == "__main__":
    rng = np.random.default_rng(0)
    edges = rng.standard_normal((B, N, N, DE), dtype=np.float32)
    nodes = rng.standard_normal((B, N, DN), dtype=np.float32)
    g_node = np.ones(DN, np.float32)
    g_edge = np.ones(DE, np.float32)
    W = rng.standard_normal((2 * DN + DE, DE), dtype=np.float32) / 18.0
    b = (rng.standard_normal(DE) * 0.01).astype(np.float32)
    o = kernel(edges, nodes, g_node, g_edge, W, b)
    print(o.shape, o.dtype)
